# revision 14
# baseline (speedup 1.0000x reference)
"""DeepSeek MoE router kernel for 8x TRN2 NeuronCores (Bass/Tile).

For x[4,8192,2048], gate_w[64,2048], expert_bias[64], noise[32768,64]:
  logits = x @ gate_w.T + noise*0.01 + expert_bias
  probs = softmax(logits); top-8 weights (renormalized) + indices;
  per-expert counts -> load-balance bias update.

Sharding: tokens (32768) split 8 ways, gate_w/expert_bias replicated,
per-expert counts summed on host (8x64 floats), bias update on host.

GEMM: x and gate_w are split hi/lo into bf16 pairs on the host
(x = xh + xl exactly to ~2^-17 relative), and the device computes
  logits = xh@wh + xl@wh + xh@wl      (bf16 matmuls, fp32 accumulate)
which carries ~fp32-grade precision. The hi/lo x shards are pre-tiled
[KCH, T, 128] so each [512, 128] block DMA-transposes (hardware xbar,
2-byte dtype) straight into the [128 K, 512 token] layout the PE needs.
This removes all PE-side transposes and PSUM->SBUF staging copies.

Stationary weights are [wh | wl] (128 cols): per k-chunk, matmul 1
computes xh against both halves into PSUM[0:128), matmul 2 accumulates
xl@wh into PSUM[0:64). logitsT = PSUM[0:64] + PSUM[64:128] + bias is a
single DVE scalar_tensor_tensor. A small PE transpose brings logitsT
back to token-major for the softmax/top-8 stage:
  DVE: +noise, row-max, reciprocals, max8/max_index8, threshold mask
  ACT: exp (with accumulated row sum), probs scale
  PE:  per-expert counts via ones.T @ (probs >= 8th value) accumulation.
"""
import sys

sys.path.insert(0, "/opt/trn_rl_repo")

import numpy as np
import ml_dtypes

import concourse.bass as bass
import concourse.tile as tile
from concourse import bacc
from concourse import mybir
from concourse import bass_utils

F32 = mybir.dt.float32
BF16 = mybir.dt.bfloat16
U32 = mybir.dt.uint32

HIDDEN = 2048
NUM_EXPERTS = 64
TOP_K = 8
JITTER = 0.01
BIAS_UPDATE_RATE = 0.001
T_TOTAL = 32768
N_CORES = 8
T_CORE = T_TOTAL // N_CORES          # 4096 tokens per core
BLK = 512                            # tokens per block
N_BLK = T_CORE // BLK                # 8 blocks
KCH = HIDDEN // 128                  # 16 k-chunks

_CACHE = {}


def _build_nc():
    nc = bacc.Bacc("TRN2", target_bir_lowering=False, debug=False)

    XHT = nc.dram_tensor("XHT", [HIDDEN, T_CORE], BF16, kind="ExternalInput").ap()
    XLT = nc.dram_tensor("XLT", [HIDDEN, T_CORE], BF16, kind="ExternalInput").ap()
    WS = nc.dram_tensor("WS", [HIDDEN, 128], BF16, kind="ExternalInput").ap()
    NZ = nc.dram_tensor("NZ", [N_BLK, 128, 4 * NUM_EXPERTS], F32, kind="ExternalInput").ap()
    BIAS = nc.dram_tensor("BIAS", [NUM_EXPERTS, 1], F32, kind="ExternalInput").ap()
    IDEN = nc.dram_tensor("IDEN", [NUM_EXPERTS, NUM_EXPERTS], F32, kind="ExternalInput").ap()
    ONES = nc.dram_tensor("ONES", [128, 1], BF16, kind="ExternalInput").ap()

    PROBS = nc.dram_tensor("PROBS", [N_BLK, 128, 4 * NUM_EXPERTS], F32, kind="ExternalOutput").ap()
    W8 = nc.dram_tensor("W8", [N_BLK, 128, 4 * TOP_K], F32, kind="ExternalOutput").ap()
    I8 = nc.dram_tensor("I8", [N_BLK, 128, 4 * TOP_K], U32, kind="ExternalOutput").ap()
    COUNTS = nc.dram_tensor("COUNTS", [1, NUM_EXPERTS], F32, kind="ExternalOutput").ap()

    with tile.TileContext(nc) as tc:
        with (
            tc.tile_pool(name="const", bufs=1) as constp,
            tc.tile_pool(name="xt", bufs=8) as xtp,
            tc.tile_pool(name="nzp", bufs=3) as nzp,
            tc.tile_pool(name="lts", bufs=2) as ltsp,
            tc.tile_pool(name="lsb", bufs=2) as lsbp,
            tc.tile_pool(name="esb", bufs=2) as esbp,
            tc.tile_pool(name="psb", bufs=N_BLK) as psbp,
            tc.tile_pool(name="small", bufs=N_BLK) as smallp,
            tc.tile_pool(name="ps_p1", bufs=2, space="PSUM") as ps_p1,
            tc.tile_pool(name="ps_bt", bufs=2, space="PSUM") as ps_bt,
            tc.tile_pool(name="ps_ct", bufs=1, space="PSUM") as ps_ct,
        ):
            # ---- preamble: constants (plain DMAs on scalar engine) -------
            iden = constp.tile([NUM_EXPERTS, NUM_EXPERTS], F32)
            nc.scalar.dma_start(iden[:], IDEN)
            ones = constp.tile([128, 1], BF16)
            nc.scalar.dma_start(ones[:], ONES)
            ws_sb = constp.tile([128, KCH, 128], BF16)
            nc.scalar.dma_start(ws_sb[:], WS.rearrange("(c p) e -> p c e", p=128))
            bias_sb = constp.tile([NUM_EXPERTS, 1], F32)
            nc.scalar.dma_start(bias_sb[:], BIAS)

            counts_ps = ps_ct.tile([1, NUM_EXPERTS], F32)

            for b in range(N_BLK):
                bsl = slice(b * BLK, (b + 1) * BLK)

                nz_sb = nzp.tile([128, 4, NUM_EXPERTS], F32, tag="nz")
                nc.scalar.dma_start(
                    nz_sb[:], NZ[b].rearrange("p (t e) -> p t e", e=NUM_EXPERTS)
                )

                # ---- GEMM: logitsT over 16 k-chunks ----------------------
                # one big load per block per hi/lo: [128, KCH, BLK] bf16 (2 MB)
                xh_t = xtp.tile([128, KCH, BLK], BF16, tag="xt", name=f"xh_{b}")
                nc.sync.dma_start(
                    xh_t[:], XHT[:, bsl].rearrange("(c p) t -> p c t", p=128)
                )
                xl_t = xtp.tile([128, KCH, BLK], BF16, tag="xt", name=f"xl_{b}")
                nc.gpsimd.dma_start(
                    xl_t[:], XLT[:, bsl].rearrange("(c p) t -> p c t", p=128)
                )
                # two PSUM accumulators so consecutive matmuls alternate
                # banks (fill overlaps drain): A += xh@[wh|wl], B += xl@wh
                pa = ps_p1.tile([128, BLK], F32, tag="pa")
                pb = ps_p1.tile([NUM_EXPERTS, BLK], F32, tag="pb")
                for kc in range(KCH):
                    nc.tensor.matmul(
                        pa[:], ws_sb[:, kc, :], xh_t[:, kc, :],
                        start=(kc == 0), stop=(kc == KCH - 1),
                        skip_group_check=True,
                    )
                    nc.tensor.matmul(
                        pb[:], ws_sb[:, kc, 0:NUM_EXPERTS], xl_t[:, kc, :],
                        start=(kc == 0), stop=(kc == KCH - 1),
                        skip_group_check=True,
                    )

                # logitsT = A[hi] + A[cross] + B + bias
                cr_sb = ltsp.tile([NUM_EXPERTS, BLK], F32, tag="crsb")
                nc.scalar.activation(
                    cr_sb[:], pa[NUM_EXPERTS : 2 * NUM_EXPERTS, :],
                    mybir.ActivationFunctionType.Copy,
                )
                t1_sb = ltsp.tile([NUM_EXPERTS, BLK], F32, tag="t1sb")
                nc.vector.scalar_tensor_tensor(
                    out=t1_sb[:],
                    in0=pa[0:NUM_EXPERTS, :],
                    scalar=bias_sb[:, 0:1],
                    in1=cr_sb[:],
                    op0=mybir.AluOpType.add,
                    op1=mybir.AluOpType.add,
                )
                lt_sb = ltsp.tile([NUM_EXPERTS, BLK], F32, tag="ltsb")
                nc.vector.tensor_add(lt_sb[:], pb[:], t1_sb[:])

                # back-transpose to token-major [128, 4*64]
                bt_ps = ps_bt.tile([128, 4 * NUM_EXPERTS], F32, tag="bt")
                for t in range(4):
                    nc.tensor.transpose(
                        bt_ps[:, t * NUM_EXPERTS : (t + 1) * NUM_EXPERTS],
                        lt_sb[:, t * 128 : (t + 1) * 128],
                        iden[:],
                    )

                # ---- logits = psum + noise*J (DVE) -----------------------
                l_sb = lsbp.tile([128, 4, NUM_EXPERTS], F32, tag="lsb")
                nc.vector.tensor_add(
                    l_sb[:],
                    bt_ps[:].rearrange("p (t e) -> p t e", e=NUM_EXPERTS),
                    nz_sb[:],
                )

                # ---- softmax + top8 --------------------------------------
                nmax = smallp.tile([128, 4], F32, tag="nmax")
                for t in range(4):
                    nc.vector.tensor_reduce(
                        out=nmax[:, t : t + 1], in_=l_sb[:, t, :],
                        op=mybir.AluOpType.max, axis=mybir.AxisListType.X,
                        negate=True,
                    )
                e_sb = esbp.tile([128, 4, NUM_EXPERTS], F32, tag="esb")
                sume = smallp.tile([128, 4], F32, tag="sume")
                for t in range(4):
                    nc.scalar.activation(
                        e_sb[:, t, :], l_sb[:, t, :],
                        mybir.ActivationFunctionType.Exp,
                        bias=nmax[:, t : t + 1], scale=1.0,
                        accum_out=sume[:, t : t + 1],
                    )
                rsum = smallp.tile([128, 4], F32, tag="rsum")
                nc.vector.reciprocal(rsum[:], sume[:])
                p_sb = psbp.tile([128, 4, NUM_EXPERTS], F32, tag="psb")
                for t in range(4):
                    nc.scalar.activation(
                        p_sb[:, t, :], e_sb[:, t, :],
                        mybir.ActivationFunctionType.Copy,
                        scale=rsum[:, t : t + 1],
                    )

                top8 = smallp.tile([128, 4, TOP_K], F32, tag="top8")
                idx8 = smallp.tile([128, 4, TOP_K], U32, tag="idx8")
                for t in range(4):
                    nc.vector.max(top8[:, t, :], p_sb[:, t, :])
                for t in range(4):
                    nc.vector.max_index(idx8[:, t, :], top8[:, t, :], p_sb[:, t, :])

                s8 = smallp.tile([128, 4], F32, tag="s8")
                for t in range(4):
                    nc.vector.tensor_reduce(
                        out=s8[:, t : t + 1], in_=top8[:, t, :],
                        op=mybir.AluOpType.add, axis=mybir.AxisListType.X,
                    )
                r8 = smallp.tile([128, 4], F32, tag="r8")
                nc.vector.reciprocal(r8[:], s8[:])
                w8_sb = smallp.tile([128, 4, TOP_K], F32, tag="w8")
                for t in range(4):
                    nc.vector.tensor_scalar_mul(
                        w8_sb[:, t, :], top8[:, t, :], r8[:, t : t + 1]
                    )

                # ---- counts: mask = probs >= 8th value; PE accumulates ---
                mask = smallp.tile([128, 4, NUM_EXPERTS], BF16, tag="mask")
                for t in range(4):
                    nc.vector.tensor_scalar(
                        out=mask[:, t, :], in0=p_sb[:, t, :],
                        scalar1=top8[:, t, TOP_K - 1 : TOP_K], scalar2=None,
                        op0=mybir.AluOpType.is_ge,
                    )
                for t in range(4):
                    nc.tensor.matmul(
                        counts_ps[:], ones[:], mask[:, t, :],
                        start=(b == 0 and t == 0),
                        stop=(b == N_BLK - 1 and t == 3),
                    )

                # ---- outputs (plain DMAs on scalar engine) ---------------
                nc.scalar.dma_start(
                    PROBS[b].rearrange("p (t e) -> p t e", e=NUM_EXPERTS), p_sb[:]
                )
                nc.scalar.dma_start(
                    W8[b].rearrange("p (t e) -> p t e", e=TOP_K), w8_sb[:]
                )
                nc.scalar.dma_start(
                    I8[b].rearrange("p (t e) -> p t e", e=TOP_K), idx8[:]
                )

            counts_sb = constp.tile([1, NUM_EXPERTS], F32)
            nc.vector.tensor_copy(counts_sb[:], counts_ps[:])
            nc.scalar.dma_start(COUNTS, counts_sb[:])

    nc.compile()
    return nc


def kernel(x, gate_w, expert_bias, noise):
    x = np.asarray(x, dtype=np.float32)
    gate_w = np.asarray(gate_w, dtype=np.float32)
    expert_bias = np.asarray(expert_bias, dtype=np.float32)
    noise = np.asarray(noise, dtype=np.float32)

    if "nc" not in _CACHE:
        _CACHE["nc"] = _build_nc()
    nc = _CACHE["nc"]

    flat_x = np.ascontiguousarray(x.reshape(T_TOTAL, HIDDEN))
    nz = noise * np.float32(JITTER)
    # device-native noise layout: [core][blk, 128, 4*64]
    nz_dev = np.ascontiguousarray(
        nz.reshape(N_CORES, N_BLK, 4, 128, NUM_EXPERTS)
        .transpose(0, 1, 3, 2, 4)
        .reshape(N_CORES, N_BLK, 128, 4 * NUM_EXPERTS)
    )

    # hi/lo bf16 split of x and gate_w.T
    xh = flat_x.astype(ml_dtypes.bfloat16)
    xl = (flat_x - xh.astype(np.float32)).astype(ml_dtypes.bfloat16)
    wt = np.ascontiguousarray(gate_w.T)                      # [H, E]
    wh = wt.astype(ml_dtypes.bfloat16)
    wl = (wt - wh.astype(np.float32)).astype(ml_dtypes.bfloat16)
    ws = np.ascontiguousarray(np.concatenate([wh, wl], axis=1))  # [H, 128]

    bias_col = np.ascontiguousarray(expert_bias[:, None])
    iden = np.eye(NUM_EXPERTS, dtype=np.float32)
    ones = np.ones((128, 1), dtype=ml_dtypes.bfloat16)

    in_maps = []
    for c in range(N_CORES):
        sl = slice(c * T_CORE, (c + 1) * T_CORE)
        xh_c = np.ascontiguousarray(xh[sl].T)
        xl_c = np.ascontiguousarray(xl[sl].T)
        in_maps.append(
            {
                "XHT": xh_c,
                "XLT": xl_c,
                "WS": ws,
                "NZ": nz_dev[c],
                "BIAS": bias_col,
                "IDEN": iden,
                "ONES": ones,
            }
        )

    _CACHE["last_in_maps"] = in_maps
    res = bass_utils.run_bass_kernel_spmd(nc, in_maps, core_ids=list(range(N_CORES)))
    outs = res.results

    def unshuffle(name, width, dt):
        a = np.stack([o[name] for o in outs])          # [C, B, 128, 4*w]
        a = a.reshape(N_CORES, N_BLK, 128, 4, width).transpose(0, 1, 3, 2, 4)
        return np.ascontiguousarray(a.reshape(T_TOTAL, width)).view(dt)

    probs = unshuffle("PROBS", NUM_EXPERTS, np.float32)
    weights = unshuffle("W8", TOP_K, np.float32)
    indices = unshuffle("I8", TOP_K, np.int32)
    counts = np.sum([o["COUNTS"][0] for o in outs], axis=0, dtype=np.float32)

    load = counts / np.float32(T_TOTAL * TOP_K)
    error = load - np.float32(1.0 / NUM_EXPERTS)
    new_bias = expert_bias - np.float32(BIAS_UPDATE_RATE) * np.sign(error, dtype=np.float32)

    return weights, indices, probs, new_bias


# revision 16
# speedup vs baseline: 1.1273x; 1.1273x over previous
"""DeepSeek MoE router kernel for 8x TRN2 NeuronCores (Bass/Tile).

For x[4,8192,2048], gate_w[64,2048], expert_bias[64], noise[32768,64]:
  logits = x @ gate_w.T + noise*0.01 + expert_bias
  probs = softmax(logits); top-8 weights (renormalized) + indices;
  per-expert counts -> load-balance bias update.

Sharding: tokens (32768) split 8 ways, gate_w/expert_bias replicated,
per-expert counts summed on host (8x64 floats), bias update on host.

GEMM: x and gate_w are split hi/lo into bf16 pairs on the host
(x = xh + xl exactly to ~2^-17 relative), and the device computes
  logits = xh@wh + xl@wh + xh@wl      (bf16 matmuls, fp32 accumulate)
which carries ~fp32-grade precision. The hi/lo x shards are pre-tiled
[KCH, T, 128] so each [512, 128] block DMA-transposes (hardware xbar,
2-byte dtype) straight into the [128 K, 512 token] layout the PE needs.
This removes all PE-side transposes and PSUM->SBUF staging copies.

Stationary weights are [wh | wl] (128 cols): per k-chunk, matmul 1
computes xh against both halves into PSUM[0:128), matmul 2 accumulates
xl@wh into PSUM[0:64). logitsT = PSUM[0:64] + PSUM[64:128] + bias is a
single DVE scalar_tensor_tensor. A small PE transpose brings logitsT
back to token-major for the softmax/top-8 stage:
  DVE: +noise, row-max, reciprocals, max8/max_index8, threshold mask
  ACT: exp (with accumulated row sum), probs scale
  PE:  per-expert counts via ones.T @ (probs >= 8th value) accumulation.
"""
import sys

sys.path.insert(0, "/opt/trn_rl_repo")

import numpy as np
import ml_dtypes

import concourse.bass as bass
import concourse.tile as tile
from concourse import bacc
from concourse import mybir
from concourse import bass_utils

F32 = mybir.dt.float32
BF16 = mybir.dt.bfloat16
U32 = mybir.dt.uint32

HIDDEN = 2048
NUM_EXPERTS = 64
TOP_K = 8
JITTER = 0.01
BIAS_UPDATE_RATE = 0.001
T_TOTAL = 32768
N_CORES = 8
T_CORE = T_TOTAL // N_CORES          # 4096 tokens per core
BLK = 512                            # tokens per block
N_BLK = T_CORE // BLK                # 8 blocks
KCH = HIDDEN // 128                  # 16 k-chunks

_CACHE = {}



def _build_nc():
    nc = bacc.Bacc("TRN2", target_bir_lowering=False, debug=False)

    XHT = nc.dram_tensor("XHT", [HIDDEN, T_CORE], BF16, kind="ExternalInput").ap()
    XLT = nc.dram_tensor("XLT", [HIDDEN, T_CORE], BF16, kind="ExternalInput").ap()
    WS = nc.dram_tensor("WS", [HIDDEN, 128], BF16, kind="ExternalInput").ap()
    NZ = nc.dram_tensor("NZ", [N_BLK, 128, 4 * NUM_EXPERTS], F32, kind="ExternalInput").ap()
    BIAS = nc.dram_tensor("BIAS", [NUM_EXPERTS, 1], F32, kind="ExternalInput").ap()
    IDEN = nc.dram_tensor("IDEN", [NUM_EXPERTS, NUM_EXPERTS], F32, kind="ExternalInput").ap()
    ONES = nc.dram_tensor("ONES", [128, 1], BF16, kind="ExternalInput").ap()

    PROBS = nc.dram_tensor("PROBS", [N_BLK, 128, 4 * NUM_EXPERTS], F32, kind="ExternalOutput").ap()
    W8 = nc.dram_tensor("W8", [N_BLK, 128, 4 * TOP_K], F32, kind="ExternalOutput").ap()
    I8 = nc.dram_tensor("I8", [N_BLK, 128, 4 * TOP_K], U32, kind="ExternalOutput").ap()
    COUNTS = nc.dram_tensor("COUNTS", [1, NUM_EXPERTS], F32, kind="ExternalOutput").ap()

    with tile.TileContext(nc) as tc:
        with (
            tc.tile_pool(name="const", bufs=1) as constp,
            tc.tile_pool(name="xt", bufs=12) as xtp,
            tc.tile_pool(name="nzp", bufs=3) as nzp,
            tc.tile_pool(name="lts", bufs=2) as ltsp,
            tc.tile_pool(name="lsb", bufs=2) as lsbp,
            tc.tile_pool(name="esb", bufs=2) as esbp,
            tc.tile_pool(name="psb", bufs=N_BLK) as psbp,
            tc.tile_pool(name="small", bufs=N_BLK) as smallp,
            tc.tile_pool(name="ps_p1", bufs=2, space="PSUM") as ps_p1,
            tc.tile_pool(name="ps_bt", bufs=2, space="PSUM") as ps_bt,
            tc.tile_pool(name="ps_ct", bufs=1, space="PSUM") as ps_ct,
        ):
            # ---- preamble: constants (plain DMAs on scalar engine) -------
            iden = constp.tile([NUM_EXPERTS, NUM_EXPERTS], F32)
            nc.scalar.dma_start(iden[:], IDEN)
            ones = constp.tile([128, 1], BF16)
            nc.scalar.dma_start(ones[:], ONES)
            ws_sb = constp.tile([128, KCH, 128], BF16)
            nc.scalar.dma_start(ws_sb[:], WS.rearrange("(c p) e -> p c e", p=128))
            bias_sb = constp.tile([NUM_EXPERTS, 1], F32)
            nc.scalar.dma_start(bias_sb[:], BIAS)

            counts_ps = ps_ct.tile([1, NUM_EXPERTS], F32)

            for b in range(N_BLK):
                bsl = slice(b * BLK, (b + 1) * BLK)

                nz_sb = nzp.tile([128, 4, NUM_EXPERTS], F32, tag="nz")
                nc.scalar.dma_start(
                    nz_sb[:], NZ[b].rearrange("p (t e) -> p t e", e=NUM_EXPERTS)
                )

                # ---- GEMM: logitsT over 16 k-chunks ----------------------
                # four 1MB half-loads per block, split across SP HWDGE and
                # POOL SWDGE queues; matmuls start as soon as each half lands
                KH = KCH // 2
                xh_a = xtp.tile([128, KH, BLK], BF16, tag="xt", name=f"xha_{b}")
                nc.sync.dma_start(
                    xh_a[:],
                    XHT[: KH * 128, bsl].rearrange("(c p) t -> p c t", p=128),
                )
                xh_b = xtp.tile([128, KH, BLK], BF16, tag="xt", name=f"xhb_{b}")
                nc.gpsimd.dma_start(
                    xh_b[:],
                    XHT[KH * 128 :, bsl].rearrange("(c p) t -> p c t", p=128),
                )
                xl_a = xtp.tile([128, KH, BLK], BF16, tag="xt", name=f"xla_{b}")
                nc.gpsimd.dma_start(
                    xl_a[:],
                    XLT[: KH * 128, bsl].rearrange("(c p) t -> p c t", p=128),
                )
                xl_b = xtp.tile([128, KH, BLK], BF16, tag="xt", name=f"xlb_{b}")
                nc.sync.dma_start(
                    xl_b[:],
                    XLT[KH * 128 :, bsl].rearrange("(c p) t -> p c t", p=128),
                )
                # single accumulator; mm pairs share one stationary [wh|wl]
                # (walrus ldw-opt dedups the reload) and xl@wl rides free.
                pa = ps_p1.tile([128, BLK], F32, tag="pa")
                for kc in range(KCH):
                    xh_m = xh_a[:, kc, :] if kc < KH else xh_b[:, kc - KH, :]
                    xl_m = xl_a[:, kc, :] if kc < KH else xl_b[:, kc - KH, :]
                    nc.tensor.matmul(
                        pa[:], ws_sb[:, kc, :], xh_m,
                        start=(kc == 0), stop=False, skip_group_check=True,
                    )
                    nc.tensor.matmul(
                        pa[:], ws_sb[:, kc, :], xl_m,
                        start=False, stop=(kc == KCH - 1), skip_group_check=True,
                    )

                # logitsT = A[hi] + A[cross] + bias
                cr_sb = ltsp.tile([NUM_EXPERTS, BLK], F32, tag="crsb")
                nc.scalar.activation(
                    cr_sb[:], pa[NUM_EXPERTS : 2 * NUM_EXPERTS, :],
                    mybir.ActivationFunctionType.Copy,
                )
                lt_sb = ltsp.tile([NUM_EXPERTS, BLK], F32, tag="ltsb")
                nc.vector.scalar_tensor_tensor(
                    out=lt_sb[:],
                    in0=pa[0:NUM_EXPERTS, :],
                    scalar=bias_sb[:, 0:1],
                    in1=cr_sb[:],
                    op0=mybir.AluOpType.add,
                    op1=mybir.AluOpType.add,
                )

                # back-transpose to token-major [128, 4*64]
                bt_ps = ps_bt.tile([128, 4 * NUM_EXPERTS], F32, tag="bt")
                for t in range(4):
                    nc.tensor.transpose(
                        bt_ps[:, t * NUM_EXPERTS : (t + 1) * NUM_EXPERTS],
                        lt_sb[:, t * 128 : (t + 1) * 128],
                        iden[:],
                    )

                # ---- logits = psum + noise*J (DVE) -----------------------
                l_sb = lsbp.tile([128, 4, NUM_EXPERTS], F32, tag="lsb")
                nc.vector.tensor_add(
                    l_sb[:],
                    bt_ps[:].rearrange("p (t e) -> p t e", e=NUM_EXPERTS),
                    nz_sb[:],
                )

                # ---- softmax + top8 --------------------------------------
                nmax = smallp.tile([128, 4], F32, tag="nmax")
                for t in range(4):
                    nc.vector.tensor_reduce(
                        out=nmax[:, t : t + 1], in_=l_sb[:, t, :],
                        op=mybir.AluOpType.max, axis=mybir.AxisListType.X,
                        negate=True,
                    )
                e_sb = esbp.tile([128, 4, NUM_EXPERTS], F32, tag="esb")
                sume = smallp.tile([128, 4], F32, tag="sume")
                for t in range(4):
                    nc.scalar.activation(
                        e_sb[:, t, :], l_sb[:, t, :],
                        mybir.ActivationFunctionType.Exp,
                        bias=nmax[:, t : t + 1], scale=1.0,
                        accum_out=sume[:, t : t + 1],
                    )
                rsum = smallp.tile([128, 4], F32, tag="rsum")
                nc.vector.reciprocal(rsum[:], sume[:])
                p_sb = psbp.tile([128, 4, NUM_EXPERTS], F32, tag="psb")
                for t in range(4):
                    nc.scalar.activation(
                        p_sb[:, t, :], e_sb[:, t, :],
                        mybir.ActivationFunctionType.Copy,
                        scale=rsum[:, t : t + 1],
                    )

                top8 = smallp.tile([128, 4, TOP_K], F32, tag="top8")
                idx8 = smallp.tile([128, 4, TOP_K], U32, tag="idx8")
                for t in range(4):
                    nc.vector.max(top8[:, t, :], p_sb[:, t, :])
                for t in range(4):
                    nc.vector.max_index(idx8[:, t, :], top8[:, t, :], p_sb[:, t, :])

                s8 = smallp.tile([128, 4], F32, tag="s8")
                for t in range(4):
                    nc.vector.tensor_reduce(
                        out=s8[:, t : t + 1], in_=top8[:, t, :],
                        op=mybir.AluOpType.add, axis=mybir.AxisListType.X,
                    )
                r8 = smallp.tile([128, 4], F32, tag="r8")
                nc.vector.reciprocal(r8[:], s8[:])
                w8_sb = smallp.tile([128, 4, TOP_K], F32, tag="w8")
                for t in range(4):
                    nc.vector.tensor_scalar_mul(
                        w8_sb[:, t, :], top8[:, t, :], r8[:, t : t + 1]
                    )

                # ---- counts: mask = probs >= 8th value; PE accumulates ---
                mask = smallp.tile([128, 4, NUM_EXPERTS], BF16, tag="mask")
                for t in range(4):
                    nc.vector.tensor_scalar(
                        out=mask[:, t, :], in0=p_sb[:, t, :],
                        scalar1=top8[:, t, TOP_K - 1 : TOP_K], scalar2=None,
                        op0=mybir.AluOpType.is_ge,
                    )
                for t in range(4):
                    nc.tensor.matmul(
                        counts_ps[:], ones[:], mask[:, t, :],
                        start=(b == 0 and t == 0),
                        stop=(b == N_BLK - 1 and t == 3),
                    )

                # ---- outputs (plain DMAs on scalar engine) ---------------
                nc.scalar.dma_start(
                    PROBS[b].rearrange("p (t e) -> p t e", e=NUM_EXPERTS), p_sb[:]
                )
                nc.scalar.dma_start(
                    W8[b].rearrange("p (t e) -> p t e", e=TOP_K), w8_sb[:]
                )
                nc.scalar.dma_start(
                    I8[b].rearrange("p (t e) -> p t e", e=TOP_K), idx8[:]
                )

            counts_sb = constp.tile([1, NUM_EXPERTS], F32)
            nc.vector.tensor_copy(counts_sb[:], counts_ps[:])
            nc.scalar.dma_start(COUNTS, counts_sb[:])

    nc.compile()
    return nc


def kernel(x, gate_w, expert_bias, noise):
    x = np.asarray(x, dtype=np.float32)
    gate_w = np.asarray(gate_w, dtype=np.float32)
    expert_bias = np.asarray(expert_bias, dtype=np.float32)
    noise = np.asarray(noise, dtype=np.float32)

    if "nc" not in _CACHE:
        _CACHE["nc"] = _build_nc()
    nc = _CACHE["nc"]

    flat_x = np.ascontiguousarray(x.reshape(T_TOTAL, HIDDEN))
    nz = noise * np.float32(JITTER)
    # device-native noise layout: [core][blk, 128, 4*64]
    nz_dev = np.ascontiguousarray(
        nz.reshape(N_CORES, N_BLK, 4, 128, NUM_EXPERTS)
        .transpose(0, 1, 3, 2, 4)
        .reshape(N_CORES, N_BLK, 128, 4 * NUM_EXPERTS)
    )

    # hi/lo bf16 split of x and gate_w.T
    xh = flat_x.astype(ml_dtypes.bfloat16)
    xl = (flat_x - xh.astype(np.float32)).astype(ml_dtypes.bfloat16)
    wt = np.ascontiguousarray(gate_w.T)                      # [H, E]
    wh = wt.astype(ml_dtypes.bfloat16)
    wl = (wt - wh.astype(np.float32)).astype(ml_dtypes.bfloat16)
    ws = np.ascontiguousarray(np.concatenate([wh, wl], axis=1))  # [H, 128]

    bias_col = np.ascontiguousarray(expert_bias[:, None])
    iden = np.eye(NUM_EXPERTS, dtype=np.float32)
    ones = np.ones((128, 1), dtype=ml_dtypes.bfloat16)

    in_maps = []
    for c in range(N_CORES):
        sl = slice(c * T_CORE, (c + 1) * T_CORE)
        xh_c = np.ascontiguousarray(xh[sl].T)
        xl_c = np.ascontiguousarray(xl[sl].T)
        in_maps.append(
            {
                "XHT": xh_c,
                "XLT": xl_c,
                "WS": ws,
                "NZ": nz_dev[c],
                "BIAS": bias_col,
                "IDEN": iden,
                "ONES": ones,
            }
        )

    _CACHE["last_in_maps"] = in_maps
    res = bass_utils.run_bass_kernel_spmd(nc, in_maps, core_ids=list(range(N_CORES)))
    outs = res.results

    def unshuffle(name, width, dt):
        a = np.stack([o[name] for o in outs])          # [C, B, 128, 4*w]
        a = a.reshape(N_CORES, N_BLK, 128, 4, width).transpose(0, 1, 3, 2, 4)
        return np.ascontiguousarray(a.reshape(T_TOTAL, width)).view(dt)

    probs = unshuffle("PROBS", NUM_EXPERTS, np.float32)
    weights = unshuffle("W8", TOP_K, np.float32)
    indices = unshuffle("I8", TOP_K, np.int32)
    counts = np.sum([o["COUNTS"][0] for o in outs], axis=0, dtype=np.float32)

    load = counts / np.float32(T_TOTAL * TOP_K)
    error = load - np.float32(1.0 / NUM_EXPERTS)
    new_bias = expert_bias - np.float32(BIAS_UPDATE_RATE) * np.sign(error, dtype=np.float32)

    return weights, indices, probs, new_bias
